# revision 2
# baseline (speedup 1.0000x reference)
"""LBLHighwayBiLm TRN2 kernel: 5-tap windowed convs + 2-layer highway nets.

Sharding: batch 32 -> 4 batches per core x 8 cores (data parallel).
Layout: activations kept [D on partitions, tokens on free] so the window
convs are free-axis-shifted fused DVE ops and highway layers chain with
no transposes. Host pre-transposes x, post-transposes the output.
Engines: PE bf16 matmuls (1 cyc/row), DVE conv taps + highway blend,
ACT sigmoid/relu(+bias) evacuating PSUM.
"""
import sys
sys.path.insert(0, '/opt/trn_rl_repo')
import numpy as np
import ml_dtypes

import concourse.bass as bass
import concourse.mybir as mybir
from concourse.bass_utils import run_bass_kernel_spmd
from concourse.tile import TileContext

BF16 = ml_dtypes.bfloat16
B, L, D, W, NL = 32, 1024, 512, 4, 2
NCORES = 8
BPC = B // NCORES            # batches per core
LP = L + 2 * W               # padded length per batch (1032)
TB = 512                     # token block
NC4 = D // 128               # 4 d-chunks
NJ = (2 * D) // 128          # 8 n-chunks per layer output


def _legalize_waits(nc):
    """walrus in this env allows ~1 sync wait per compute/DMA instruction;
    split extras into standalone EventSemaphore nops on the same engine."""
    CAPS = {"InstDMACopy": 1, "InstMatmult": 1, "InstTensorCopy": 1,
            "InstTensorScalarPtr": 1, "InstTensorTensor": 1,
            "InstActivation": 1, "InstMemset": 1, "InstTensorReduce": 1,
            "InstDrain": 1, "InstIota": 1}
    k = 0
    for f in nc.m.functions:
        for bb in f.blocks:
            newlist = []
            for ins in bb.instructions:
                si = ins.sync_info
                cap = CAPS.get(type(ins).__name__)
                if si is not None and cap is not None and len(si.on_wait) > cap:
                    waits = list(si.on_wait)
                    extras, keep = waits[:-cap], waits[-cap:]
                    for w in extras:
                        es = mybir.InstEventSemaphore(name=f"legalw_{k}", ins=[], outs=[])
                        k += 1
                        es.engine = ins.engine
                        es.sync_info = mybir.SyncInfo(on_wait=[w], on_update=[])
                        newlist.append(es)
                    ins.sync_info = mybir.SyncInfo(on_wait=keep,
                                                   on_update=list(si.on_update))
                newlist.append(ins)
            try:
                bb.instructions = newlist
            except Exception:
                bb.instructions.clear()
                bb.instructions.extend(newlist)
    return k


def _build_nc():
    nc = bass.Bass()
    f32, bf = mybir.dt.float32, mybir.dt.bfloat16
    xt_ext = nc.declare_dram_parameter("xt", [D, BPC * LP], bf, isOutput=False)
    w_ext = {}
    for ld in range(4):  # 0=l0 1=l1 2=r0 3=r1
        w_ext[ld] = nc.declare_dram_parameter(f"w{ld}", [NC4, 128, 2 * D], bf,
                                              isOutput=False)
    bias_ext = nc.declare_dram_parameter("bias", [128, 4 * NJ], f32, isOutput=False)
    taps_ext = nc.declare_dram_parameter("taps", [128, 10], f32, isOutput=False)
    out_ext = nc.declare_dram_parameter("out", [2 * D, BPC * L], f32, isOutput=True)

    AF = mybir.ActivationFunctionType
    OP = mybir.AluOpType

    with TileContext(nc) as tc:
        with (
            tc.tile_pool(name="wp", bufs=1) as wp,
            tc.tile_pool(name="cst", bufs=1) as cst,
            tc.tile_pool(name="xin", bufs=2) as xinp,
            tc.tile_pool(name="cv", bufs=2) as cvp,
            tc.tile_pool(name="rg", bufs=2) as rgp,
            tc.tile_pool(name="x1", bufs=2) as x1p,
            tc.tile_pool(name="xo", bufs=2) as xop,
            tc.tile_pool(name="ps", bufs=4, space="PSUM") as psp,
        ):
            taps = cst.tile([128, 10], f32)
            nc.sync.dma_start(out=taps[:, :], in_=taps_ext[:, :])
            btile = cst.tile([128, 4 * NJ], f32)
            nc.sync.dma_start(out=btile[:, :], in_=bias_ext[:, :])
            wt = {}
            for ld in range(4):
                w_tile = wp.tile([128, NC4 * 2 * D], bf, tag=f"w{ld}")
                wt[ld] = w_tile
                for c in range(NC4):
                    nc.sync.dma_start(out=wt[ld][:, c * 2 * D:(c + 1) * 2 * D],
                                      in_=w_ext[ld][c])

            for b in range(BPC):
                for h in range(L // TB):
                    # ---- load x block [128, TB+8] per d-chunk ----
                    xin = []
                    for c in range(NC4):
                        xc = xinp.tile([128, TB + 2 * W], bf, tag=f"xi{c}")
                        col0 = b * LP + h * TB
                        nc.sync.dma_start(out=xc[:, :],
                                          in_=xt_ext[c * 128:(c + 1) * 128,
                                                     col0:col0 + TB + 2 * W])
                        xin.append(xc)
                    # ---- conv: left taps k..k+4 / right taps 4+k ----
                    conv = {}
                    for d_i, dname in enumerate(("l", "r")):
                        off = 0 if dname == "l" else W
                        for c in range(NC4):
                            cv = cvp.tile([128, TB], bf, tag=f"cv{dname}{c}")
                            tap0 = taps[:, 5 * d_i: 5 * d_i + 1]
                            nc.vector.tensor_scalar_mul(
                                cv[:, :], xin[c][:, off:off + TB], tap0)
                            for k in range(1, W + 1):
                                tk = taps[:, 5 * d_i + k: 5 * d_i + k + 1]
                                nc.vector.scalar_tensor_tensor(
                                    cv[:, :], xin[c][:, off + k:off + k + TB],
                                    tk, cv[:, :], OP.mult, OP.add)
                            conv[(dname, c)] = cv
                    # ---- highway layers ----
                    for d_i, dname in enumerate(("l", "r")):
                        xc = [conv[(dname, c)] for c in range(NC4)]
                        for li in range(NL):
                            ld = d_i * 2 + li
                            rg = []
                            for j in range(NJ):
                                ps = psp.tile([128, 512], f32, tag="ps")
                                for c in range(NC4):
                                    nc.tensor.matmul(
                                        ps[:, :],
                                        wt[ld][:, c * 2 * D + j * 128:
                                               c * 2 * D + j * 128 + 128],
                                        xc[c][:, :],
                                        start=(c == 0), stop=(c == NC4 - 1))
                                bcol = btile[:, ld * NJ + j: ld * NJ + j + 1]
                                t = rgp.tile([128, TB], bf, tag=f"rg{j}")
                                fn = AF.Relu if j < NC4 else AF.Sigmoid
                                nc.scalar.activation(t[:, :], ps[:, :], fn,
                                                     bias=bcol)
                                rg.append(t)
                            last = li == NL - 1
                            xnew = []
                            for c in range(NC4):
                                r, g = rg[c], rg[NC4 + c]
                                t1 = x1p.tile([128, TB], bf, tag=f"t1{c}")
                                nc.vector.tensor_tensor(
                                    t1[:, :], xc[c][:, :], r[:, :], OP.subtract)
                                nc.vector.tensor_tensor(
                                    t1[:, :], g[:, :], t1[:, :], OP.mult)
                                if last:
                                    xo = xop.tile([128, TB], f32, tag=f"xo{c}")
                                    nc.vector.tensor_tensor(
                                        xo[:, :], t1[:, :], r[:, :], OP.add)
                                    row0 = d_i * D + c * 128
                                    nc.sync.dma_start(
                                        out=out_ext[row0:row0 + 128,
                                                    b * L + h * TB:
                                                    b * L + h * TB + TB],
                                        in_=xo[:, :])
                                else:
                                    xn = x1p.tile([128, TB], bf, tag=f"xn{c}")
                                    nc.vector.tensor_tensor(
                                        xn[:, :], t1[:, :], r[:, :], OP.add)
                                    xnew.append(xn)
                            if not last:
                                xc = xnew
    _legalize_waits(nc)
    return nc


_NC_CACHE = None


def kernel(inputs, left_padding, right_padding, left_weights, right_weights,
           left_W, left_b, right_W, right_b):
    global _NC_CACHE
    inputs = np.asarray(inputs, dtype=np.float32)
    left_padding = np.asarray(left_padding, dtype=np.float32)
    right_padding = np.asarray(right_padding, dtype=np.float32)
    left_weights = np.asarray(left_weights, dtype=np.float32)
    right_weights = np.asarray(right_weights, dtype=np.float32)
    left_W = np.asarray(left_W, dtype=np.float32)
    left_b = np.asarray(left_b, dtype=np.float32)
    right_W = np.asarray(right_W, dtype=np.float32)
    right_b = np.asarray(right_b, dtype=np.float32)

    # ---- host prep ----
    pad = np.empty((B, LP, D), dtype=np.float32)
    pad[:, W:W + L] = inputs
    pad[:, :W] = left_padding
    pad[:, W + L:] = right_padding
    pad_bf = pad.astype(BF16)

    w_all = []
    for Wm in (left_W[0], left_W[1], right_W[0], right_W[1]):
        w_all.append(np.ascontiguousarray(
            Wm.astype(BF16).reshape(NC4, 128, 2 * D)))
    bias = np.empty((128, 4 * NJ), dtype=np.float32)
    for ld, bv in enumerate((left_b[0], left_b[1], right_b[0], right_b[1])):
        bias[:, ld * NJ:(ld + 1) * NJ] = bv.reshape(NJ, 128).T
    taps = np.empty((128, 10), dtype=np.float32)
    taps[:, :5] = left_weights[None, :]
    taps[:, 5:] = right_weights[None, :]

    in_maps = []
    for core in range(NCORES):
        blk = pad_bf[core * BPC:(core + 1) * BPC]          # [BPC, LP, D]
        xt = np.ascontiguousarray(
            blk.transpose(2, 0, 1).reshape(D, BPC * LP))
        m = {"xt": xt, "bias": bias, "taps": taps}
        for ld in range(4):
            m[f"w{ld}"] = w_all[ld]
        in_maps.append(m)

    if _NC_CACHE is None:
        _NC_CACHE = _build_nc()
    res = run_bass_kernel_spmd(_NC_CACHE, in_maps, list(range(NCORES)))

    last = np.empty((B, L, 2 * D), dtype=np.float32)
    for core in range(NCORES):
        o = res.results[core]["out"]                        # [2D, BPC*L]
        last[core * BPC:(core + 1) * BPC] = (
            o.reshape(2 * D, BPC, L).transpose(1, 2, 0))
    return last[None], last
